# revision 9
# baseline (speedup 1.0000x reference)
"""Trainium2 Bass kernel for nn_HTTNBaseHead (segment_reduce).

Sharding: classes (C=8192) are sharded over 8 cores, 1024 each, interleaved so
that every core owns 64 of the 512 head classes plus 960 tail classes.  Each
core segment-reduces its own label columns over the full batch (counts fused as
a ones-column in the GEMM rhs), updates its prototype shard, AllGathers the
head-class prototypes, then runs the attention + mapper chain on its shard and
produces its 1024 columns of the logits.
"""
import sys

sys.path.insert(0, "/opt/trn_rl_repo")

import numpy as np

import concourse.bacc as bacc
import concourse.mybir as mybir
import concourse.tile as tile
from concourse.bass_utils import run_bass_kernel_spmd

B, C, D, HID, H = 4096, 8192, 256, 512, 512
EMA = 0.99
N_CORES = 8
CS = C // N_CORES          # 1024 classes per core
HS = H // N_CORES          # 64 head classes per core
TS = (C - H) // N_CORES    # 960 tail classes per core
KT = B // 128              # 32 batch k-tiles
MT = CS // 128             # 8 class m-tiles
F32 = mybir.dt.float32
AF = mybir.ActivationFunctionType
ALU = mybir.AluOpType

_prog = None


def _build():
    core_ids = list(range(N_CORES))
    nc = bacc.Bacc("TRN2", target_bir_lowering=False, debug=False,
                   enable_asserts=True, num_devices=N_CORES)

    labels_d = nc.dram_tensor("labels_s", [B, CS], F32, kind="ExternalInput").ap()
    protos_d = nc.dram_tensor("protos_s", [CS, D], F32, kind="ExternalInput").ap()
    coefe_d = nc.dram_tensor("coefe", [128, MT], F32, kind="ExternalInput").ap()
    coeff_d = nc.dram_tensor("coeff", [128, MT], F32, kind="ExternalInput").ap()
    zaug_d = nc.dram_tensor("z_aug", [B, D + 1], F32, kind="ExternalInput").ap()
    zt_d = nc.dram_tensor("zT", [D, B], F32, kind="ExternalInput").ap()
    w1t_d = nc.dram_tensor("w1T", [D, HID], F32, kind="ExternalInput").ap()
    w2t_d = nc.dram_tensor("w2T", [HID, D + 1], F32, kind="ExternalInput").ap()
    b1c_d = nc.dram_tensor("b1c", [128, 4], F32, kind="ExternalInput").ap()
    b2c_d = nc.dram_tensor("b2c", [128, 3], F32, kind="ExternalInput").ap()
    ident_d = nc.inline_tensor(np.eye(128, dtype=np.float32), name="ident")
    logits_d = nc.dram_tensor("logits_s", [B, CS], F32, kind="ExternalOutput").ap()

    with tile.TileContext(nc) as tc:
        with tc.tile_pool(name="sbc", bufs=1) as sbc, \
             tc.tile_pool(name="sbt", bufs=2) as sbt:
            # ---- persistent small tensors ----
            protos = sbc.tile([128, MT * D], F32)
            nc.sync.dma_start(protos[:].rearrange("p (m d) -> p m d", m=MT), protos_d.rearrange("(m p) d -> p m d", p=128))
            coefe = sbc.tile([128, MT], F32)
            nc.sync.dma_start(coefe[:], coefe_d[:])
            coeff = sbc.tile([128, MT], F32)
            nc.sync.dma_start(coeff[:], coeff_d[:])
            w1t = sbc.tile([128, 2 * HID], F32)
            nc.sync.dma_start(w1t[:].rearrange("p (t i) -> p t i", t=2), w1t_d.rearrange("(t p) i -> p t i", p=128))
            w2t = sbc.tile([128, 4 * (D + 1)], F32)
            nc.sync.dma_start(w2t[:].rearrange("p (t j) -> p t j", t=4), w2t_d.rearrange("(t p) j -> p t j", p=128))
            b1c = sbc.tile([128, 4], F32)
            nc.sync.dma_start(b1c[:], b1c_d[:])
            b2c = sbc.tile([128, 3], F32)
            nc.sync.dma_start(b2c[:], b2c_d[:])
            ident = sbc.tile([128, 128], F32)
            nc.sync.dma_start(ident[:], ident_d.ap())
            ones_col = sbc.tile([128, 1], F32)
            nc.vector.memset(ones_col[:], 1.0)
            ones_row = sbc.tile([1, 128], F32)
            nc.vector.memset(ones_row[:], 1.0)

            Pt = sbc.tile([128, MT * D], F32)         # P, classes on partitions
            ph = sbc.tile([128, 4 * D], F32)          # P_head [512, 256]
            PT = sbc.tile([128, 2 * CS], F32)         # P^T   [256, 1024]
            PHT = sbc.tile([128, 2 * H], F32)         # P_head^T [256, 512]
            expST = sbc.tile([128, 4 * CS], F32)      # exp(scores^T) [512, 1024]
            rs_row = sbc.tile([1, CS], F32)           # 1/rowsum  [1, 1024]
            rs_b = sbc.tile([128, CS], F32)           # bcast over partitions
            pefT = sbc.tile([128, 2 * CS], F32)       # P_eff^T [256, 1024]
            hT = sbc.tile([128, 4 * CS], F32)         # h^T [512, 1024]
            WT = sbc.tile([128, 2 * CS], F32)         # W^T [256, 1024]
            brow = sbc.tile([1, CS], F32)             # per-class bias row

            # ================= stage 1: segment reduce =================
            with tc.tile_pool(name="seg_sb", bufs=1) as seg_sb, \
                 tc.tile_pool(name="lab_sb", bufs=3) as lab_sb, \
                 tc.tile_pool(name="msk_sb", bufs=2) as msk_sb, \
                 tc.tile_pool(name="seg_ps", bufs=1, space="PSUM") as seg_ps:
                zaug = seg_sb.tile([128, KT * (D + 1)], F32)
                nc.sync.dma_start(zaug[:].rearrange("p (k d) -> p k d", k=KT), zaug_d.rearrange("(k p) d -> p k d", p=128))
                SC = [seg_ps.tile([128, D + 1], F32, name=f"SC{m}") for m in range(MT)]
                for k in range(KT):
                    lab = lab_sb.tile([128, CS], F32, name="lab")
                    nc.sync.dma_start(lab[:], labels_d[k * 128:(k + 1) * 128, :])
                    msk = msk_sb.tile([128, CS], F32, name="msk")
                    nc.vector.tensor_scalar(msk[:], lab[:], 0.5, None, op0=ALU.is_gt)
                    for m in range(MT):
                        nc.tensor.matmul(
                            SC[m][:], msk[:, m * 128:(m + 1) * 128],
                            zaug[:, k * (D + 1):(k + 1) * (D + 1)],
                            start=(k == 0), stop=(k == KT - 1))

                # ---- stage 2: prototype EMA update ----
                for m in range(MT):
                    cnt = SC[m][:, D:D + 1]
                    hp = sbt.tile([128, 1], F32, name="hp")
                    nc.vector.tensor_scalar(hp[:], cnt, 0.0, None, op0=ALU.is_gt)
                    mx = sbt.tile([128, 1], F32, name="mx")
                    nc.vector.tensor_scalar_max(mx[:], cnt, 1.0)
                    scale = sbt.tile([128, 1], F32, name="scale")
                    nc.vector.reciprocal(scale[:], mx[:])
                    # coefA = 1 + hp*(e-1);  coefB = hp * f * scale
                    ca = sbt.tile([128, 1], F32, name="ca")
                    nc.vector.tensor_scalar_add(ca[:], coefe[:, m:m + 1], -1.0)
                    nc.vector.tensor_mul(ca[:], ca[:], hp[:])
                    nc.vector.tensor_scalar_add(ca[:], ca[:], 1.0)
                    cb = sbt.tile([128, 1], F32, name="cb")
                    nc.vector.tensor_mul(cb[:], coeff[:, m:m + 1], hp[:])
                    nc.vector.tensor_mul(cb[:], cb[:], scale[:])
                    pm = Pt[:, m * D:(m + 1) * D]
                    nc.vector.tensor_scalar_mul(pm, protos[:, m * D:(m + 1) * D], ca[:])
                    sm = sbt.tile([128, D], F32, name="sm")
                    nc.vector.tensor_scalar_mul(sm[:], SC[m][:, 0:D], cb[:])
                    nc.vector.tensor_add(pm, pm, sm[:])

            # ================= stage 3: AllGather P_head =================
            with tc.tile_pool(name="dramp", bufs=1, space="DRAM") as dramp, \
                 tc.tile_pool(name="zt_sb", bufs=1) as zt_sb, \
                 tc.tile_pool(name="out_sb", bufs=3) as out_sb, \
                 tc.tile_pool(name="ps2", bufs=4, space="PSUM") as ps2, \
                 tc.tile_pool(name="ps3", bufs=2, space="PSUM") as ps3:
                zt = zt_sb.tile([128, 2 * B], F32)
                nc.sync.dma_start(zt[:].rearrange("p (t b) -> p t b", t=2), zt_d.rearrange("(t p) b -> p t b", p=128))

                cc_in = dramp.tile([HS, D], F32)
                cc_out = dramp.tile([H, D], F32, addr_space="Shared")
                nc.sync.dma_start(cc_in[:], Pt[0:HS, 0:D])
                nc.gpsimd.collective_compute(
                    "AllGather", ALU.bypass,
                    replica_groups=[core_ids],
                    ins=[cc_in.opt()], outs=[cc_out.opt()])
                nc.sync.dma_start(ph[:].rearrange("p (m d) -> p m d", m=4), cc_out.opt().rearrange("(m p) d -> p m d", p=128))

                # ---- stage 4: transposes P^T, P_head^T ----
                for m in range(MT):
                    for dk in range(2):
                        tp = ps3.tile([128, 128], F32, name="tp", tag="sm")
                        nc.tensor.transpose(tp[:], Pt[:, m * D + dk * 128: m * D + (dk + 1) * 128], ident[:])
                        dst = PT[:, dk * CS + m * 128: dk * CS + (m + 1) * 128]
                        if m % 2 == 0:
                            nc.vector.tensor_copy(dst, tp[:])
                        else:
                            nc.scalar.activation(dst, tp[:], AF.Copy)
                for hm in range(4):
                    for dk in range(2):
                        tp = ps3.tile([128, 128], F32, name="tp", tag="sm")
                        nc.tensor.transpose(tp[:], ph[:, hm * D + dk * 128: hm * D + (dk + 1) * 128], ident[:])
                        dst = PHT[:, dk * H + hm * 128: dk * H + (hm + 1) * 128]
                        if hm % 2 == 0:
                            nc.vector.tensor_copy(dst, tp[:])
                        else:
                            nc.scalar.activation(dst, tp[:], AF.Copy)

                # ---- stage 5: scores^T -> exp -> row sums ----
                for hm in range(4):
                    for nch in range(2):
                        st = ps2.tile([128, 512], F32, name="st", tag="mm")
                        for dk in range(2):
                            nc.tensor.matmul(
                                st[:], PHT[:, dk * H + hm * 128: dk * H + (hm + 1) * 128],
                                PT[:, dk * CS + nch * 512: dk * CS + (nch + 1) * 512],
                                start=(dk == 0), stop=(dk == 1))
                        nc.scalar.activation(
                            expST[:, hm * CS + nch * 512: hm * CS + (nch + 1) * 512],
                            st[:], AF.Exp, scale=float(1.0 / np.sqrt(D)))
                for nch in range(2):
                    sp = ps3.tile([1, 512], F32, name="sp", tag="sm")
                    for hm in range(4):
                        nc.tensor.matmul(
                            sp[:], ones_col[:],
                            expST[:, hm * CS + nch * 512: hm * CS + (nch + 1) * 512],
                            start=(hm == 0), stop=(hm == 3))
                    nc.vector.reciprocal(rs_row[0:1, nch * 512:(nch + 1) * 512], sp[:])
                for nch in range(2):
                    rb = ps3.tile([128, 512], F32, name="rb", tag="sm")
                    nc.tensor.matmul(rb[:], ones_row[:],
                                     rs_row[0:1, nch * 512:(nch + 1) * 512],
                                     start=True, stop=True)
                    nc.vector.tensor_copy(rs_b[:, nch * 512:(nch + 1) * 512], rb[:])

                # ---- stage 6: attn^T and P_eff^T ----
                for dm in range(2):
                    for nch in range(2):
                        at = ps2.tile([128, 512], F32, name="at", tag="mm")
                        for hm in range(4):
                            nc.tensor.matmul(
                                at[:], ph[:, hm * D + dm * 128: hm * D + (dm + 1) * 128],
                                expST[:, hm * CS + nch * 512: hm * CS + (nch + 1) * 512],
                                start=(hm == 0), stop=(hm == 3))
                        sl = slice(dm * CS + nch * 512, dm * CS + (nch + 1) * 512)
                        tmp = sbt.tile([128, 512], F32, name="tmp")
                        nc.vector.tensor_mul(tmp[:], at[:], rs_b[:, nch * 512:(nch + 1) * 512])
                        nc.vector.tensor_add(pefT[:, sl], tmp[:], PT[:, sl])

                # ---- stage 7: mapper MLP (transposed) ----
                for im in range(4):
                    for nch in range(2):
                        hp2 = ps2.tile([128, 512], F32, name="hp2", tag="mm")
                        for dk in range(2):
                            nc.tensor.matmul(
                                hp2[:], w1t[:, dk * HID + im * 128: dk * HID + (im + 1) * 128],
                                pefT[:, dk * CS + nch * 512: dk * CS + (nch + 1) * 512],
                                start=(dk == 0), stop=(dk == 1))
                        nc.scalar.activation(
                            hT[:, im * CS + nch * 512: im * CS + (nch + 1) * 512],
                            hp2[:], AF.Relu, bias=b1c[:, im:im + 1])
                for jm in range(2):
                    for nch in range(2):
                        wb = ps2.tile([128, 512], F32, name="wb", tag="mm")
                        for ik in range(4):
                            nc.tensor.matmul(
                                wb[:], w2t[:, ik * (D + 1) + jm * 128: ik * (D + 1) + (jm + 1) * 128],
                                hT[:, ik * CS + nch * 512: ik * CS + (nch + 1) * 512],
                                start=(ik == 0), stop=(ik == 3))
                        nc.scalar.activation(
                            WT[:, jm * CS + nch * 512: jm * CS + (nch + 1) * 512],
                            wb[:], AF.Identity, bias=b2c[:, jm:jm + 1])
                for nch in range(2):
                    br = ps3.tile([1, 512], F32, name="br", tag="sm")
                    for ik in range(4):
                        nc.tensor.matmul(
                            br[:], w2t[:, ik * (D + 1) + D: ik * (D + 1) + D + 1],
                            hT[:, ik * CS + nch * 512: ik * CS + (nch + 1) * 512],
                            start=(ik == 0), stop=(ik == 3))
                    nc.scalar.activation(brow[0:1, nch * 512:(nch + 1) * 512],
                                         br[:], AF.Identity, bias=b2c[0:1, 2:3])

                # ---- stage 8: logits = z @ W^T + b ----
                for bt in range(KT):
                    ot = out_sb.tile([128, CS], F32, name="ot")
                    for nch in range(2):
                        lg = ps2.tile([128, 512], F32, name="lg", tag="mm")
                        nc.tensor.matmul(lg[:], zt[:, bt * 128:(bt + 1) * 128],
                                         WT[:, nch * 512:(nch + 1) * 512],
                                         start=True, stop=False)
                        nc.tensor.matmul(lg[:], zt[:, B + bt * 128: B + (bt + 1) * 128],
                                         WT[:, CS + nch * 512: CS + (nch + 1) * 512],
                                         start=False, stop=False)
                        nc.tensor.matmul(lg[:], ones_row[:],
                                         brow[0:1, nch * 512:(nch + 1) * 512],
                                         start=False, stop=True)
                        dst = ot[:, nch * 512:(nch + 1) * 512]
                        if nch == 0:
                            nc.vector.tensor_copy(dst, lg[:])
                        else:
                            nc.scalar.activation(dst, lg[:], AF.Copy)
                    nc.sync.dma_start(logits_d[bt * 128:(bt + 1) * 128, :], ot[:])

    nc.finalize()
    return nc, core_ids


def _get_prog():
    global _prog
    if _prog is None:
        _prog = _build()
    return _prog


def shard_cols(a, i):
    # class axis shard for core i: [head 64 | tail 960] columns
    hi = slice(HS * i, HS * (i + 1))
    ti = slice(H + TS * i, H + TS * (i + 1))
    return hi, ti


def prep_in_maps(z, labels, prototypes, proto_inited, head_ids, w1, b1, w2, b2):
    z = np.asarray(z, np.float32)
    labels = np.asarray(labels, np.float32)
    prototypes = np.asarray(prototypes, np.float32)
    inited = np.asarray(proto_inited).astype(bool)
    head_ids = np.asarray(head_ids)
    assert np.array_equal(head_ids, np.arange(H)), "kernel assumes head_ids == arange(H)"
    w1 = np.asarray(w1, np.float32)
    b1 = np.asarray(b1, np.float32)
    w2 = np.asarray(w2, np.float32)
    b2 = np.asarray(b2, np.float32)

    e_full = np.where(inited, np.float32(EMA), np.float32(0.0)).astype(np.float32)
    f_full = np.where(inited, np.float32(1.0 - EMA), np.float32(1.0)).astype(np.float32)
    zaug = np.concatenate([z, np.ones((B, 1), np.float32)], axis=1)
    zt = np.ascontiguousarray(z.T)
    w1t = np.ascontiguousarray(w1.T)
    w2t = np.ascontiguousarray(w2.T)
    b1c = np.ascontiguousarray(b1.reshape(4, 128).T)
    b2p = np.zeros(3 * 128, np.float32)
    b2p[:D + 1] = b2
    b2c = np.ascontiguousarray(b2p.reshape(3, 128).T)

    in_maps = []
    for i in range(N_CORES):
        hi, ti = shard_cols(labels, i)
        lab_s = np.ascontiguousarray(np.concatenate([labels[:, hi], labels[:, ti]], axis=1))
        pro_s = np.ascontiguousarray(np.concatenate([prototypes[hi], prototypes[ti]], axis=0))
        e_s = np.concatenate([e_full[hi], e_full[ti]])
        f_s = np.concatenate([f_full[hi], f_full[ti]])
        in_maps.append({
            "labels_s": lab_s,
            "protos_s": pro_s,
            "coefe": np.ascontiguousarray(e_s.reshape(MT, 128).T),
            "coeff": np.ascontiguousarray(f_s.reshape(MT, 128).T),
            "z_aug": zaug,
            "zT": zt,
            "w1T": w1t,
            "w2T": w2t,
            "b1c": b1c,
            "b2c": b2c,
        })

    return in_maps


def kernel(**inputs):
    nc, core_ids = _get_prog()
    in_maps = prep_in_maps(**inputs)
    res = run_bass_kernel_spmd(nc, in_maps, core_ids).results

    logits = np.empty((B, C), np.float32)
    for i in core_ids:
        hi, ti = shard_cols(None, i)
        out_s = res[i]["logits_s"]
        logits[:, hi] = out_s[:, :HS]
        logits[:, ti] = out_s[:, HS:]
    return logits
